# revision 12
# baseline (speedup 1.0000x reference)
"""Trainium2 Bass kernel for nn_ATTEfficient (ragged segment attention pooling).

reference:
    H = tanh(features @ Ww.T + bw)          # [TOTAL, D]
    s = H @ v                                # [TOTAL]
    att = segment_softmax(s, segment_ids)    # [TOTAL]
    pooled = segment_sum(features * att)     # [N_SEG, D]
    h = relu(pooled @ W1.T + b1)             # [N_SEG, D_HEAD]
    out = h @ W2.T + b2                      # [N_SEG, 1]

Sharding: segments (and their contiguous token ranges) split across the 8
cores data-parallel, balanced by token count; weights replicated; all
segment reductions local to one core.

Device pipeline per core, token-partition ("natural H") layout, one
128-token chunk at a time:
    - H chunk [128 tok, 1280] = X.T-pair stationary (fp8 DoubleRow, W
      pre-scaled x64 on host) x Ww.T moving, accumulated in 3 PSUM column
      groups; ACT tanh (scale 1/64) -> ht bf16
    - s row via ONE fused DVE tensor_tensor_reduce (ht * v-broadcast,
      reduce over features) -> per-token s in partition layout, ACT exp
      (no max subtraction: |s| is O(1), exp cannot overflow)
    - A = onehot_mask * e (DVE), pooling matmuls (bf16) accumulate
      pooled[seg,:]+z across all chunks into one 3-bank PSUM group
  tail: z clamp/recip, DVE 32x32 stream-transposes of pooled (PSUM->SBUF
  direct, no PE transposes), 10 bf16 head matmuls, relu, fused DVE dot
  with W2, 1/z scale, out written token-partition [S,1] (host reshapes).

fp8 is e4m3 on X and 64*Ww only (end-to-end rel err ~1.5e-2 measured in
simulation vs the 2e-2 gate); pooling/head stay bf16.

PSUM accumulators are initialized by zero-matmuls (start=True writing the
whole used range) so subsequent accumulating matmuls are order-free.
"""

import os
import numpy as np
import ml_dtypes

import concourse.bass as bass
import concourse.tile as tile
from concourse import bacc, mybir
from concourse.bass_utils import run_bass_kernel_spmd
from concourse.masks import make_identity

F32 = mybir.dt.float32
BF16 = mybir.dt.bfloat16
FP8 = mybir.dt.float8e4
AF = mybir.ActivationFunctionType
ALU = mybir.AluOpType
NPF8 = ml_dtypes.float8_e4m3
NPBF = ml_dtypes.bfloat16

N_CORES = 8
N_SEG = 128
D = 1280
KB = D // 128   # 10 feature blocks
DH = 512
SP = 32         # padded segment-partition count (DVE transpose block)
WSCALE = 64.0   # fp8 weight pre-scale (undone by ACT tanh input scale)

USE_FP8 = bool(int(os.environ.get("KERNEL_FP8", "1")))  # False -> bf16 H
TAIL_PE = bool(int(os.environ.get("KERNEL_TAIL_PE", "1")))  # PE-transpose tail

LAST_RESULTS = None
_PROGRAM_CACHE = {}


def _partition_segments(lengths: np.ndarray) -> list[int]:
    """Split N_SEG contiguous segments into N_CORES contiguous groups
    minimizing the max token count (binary search + greedy packing)."""
    lengths = lengths.astype(np.int64)
    total = int(lengths.sum())

    def cuts_for(cap):
        cuts = [0]
        cur = 0
        for i, L in enumerate(lengths):
            if cur + L > cap and cur > 0:
                cuts.append(i)
                cur = 0
                if len(cuts) > N_CORES:
                    return None
            cur += int(L)
        while len(cuts) < N_CORES:
            cuts.append(N_SEG)
        cuts.append(N_SEG)
        return cuts

    lo, hi = max(int(lengths.max()), (total + N_CORES - 1) // N_CORES), total
    while lo < hi:
        mid = (lo + hi) // 2
        if cuts_for(mid) is not None:
            hi = mid
        else:
            lo = mid + 1
    return cuts_for(lo)


def _emit(tc: tile.TileContext, t: dict, T_pad: int, S: int,
          b1_zero: bool, bw_zero: bool):
    nc = tc.nc
    NB = T_pad // 128
    GRP = ((0, 512), (512, 512), (1024, 256))  # H psum column groups
    mmdt = FP8 if USE_FP8 else BF16

    with tc.tile_pool(name="const", bufs=1) as cp:
        zo_sb = cp.tile([128, 514], BF16)
        nc.sync.dma_start(out=zo_sb[:], in_=t["zo"][:])
        wwt_sb = cp.tile([128, KB, D], mmdt)
        xt_sb = cp.tile([128, KB, T_pad], mmdt)
        vrep_sb = cp.tile([128, D], BF16)
        m1h_sb = cp.tile([128, NB, S], BF16)
        e_sb = cp.tile([128, NB], F32)
        w1t_sb = cp.tile([128, KB, DH], BF16)
        w2b_sb = cp.tile([SP, DH], BF16)
        bwrep_sb = cp.tile([128, D], F32)
        b1rep_sb = cp.tile([SP, DH], F32)

        with tc.tile_pool(name="accps", bufs=1, space="PSUM") as accp:
            # pooled accumulator: blocks 0:10 pooled (d), block 10 cols
            # 0:2 hold z
            pooled_ps = accp.tile([SP, 12, 128], F32)

            # zero-matmuls: order-safe accumulator init + HAM warm while the
            # startup weight/data DMAs stream in
            for _rep in range(6):
                for a0 in (0, 4, 8):
                    nc.tensor.matmul(pooled_ps[:, a0:a0 + 4, :],
                                     zo_sb[:, 0:SP], zo_sb[:, 0:512],
                                     start=True, stop=False,
                                     skip_group_check=True)

            with tc.tile_pool(name="hps2", bufs=2, space="PSUM") as hps2, \
                 tc.tile_pool(name="hps1", bufs=1, space="PSUM") as hps1, \
                 tc.tile_pool(name="xnp", bufs=7) as xnp, \
                 tc.tile_pool(name="htp", bufs=2) as htp, \
                 tc.tile_pool(name="prodp", bufs=2) as prodp, \
                 tc.tile_pool(name="sp", bufs=4) as sp_pool, \
                 tc.tile_pool(name="ap", bufs=3) as ap_pool, \
                 tc.tile_pool(name="xnp2", bufs=1) as _unused:

                xn_tiles = {}
                pend = {}
                spend = {}

                def emit_expA(c):
                    s_sb = spend.pop(c)
                    nc.scalar.activation(e_sb[:, c:c + 1], s_sb[:], AF.Exp)
                    A_sb = ap_pool.tile([128, S], BF16)
                    nc.vector.tensor_scalar_mul(
                        A_sb[:], m1h_sb[:, c, 0:S], e_sb[:, c:c + 1])
                    pend[c] = (A_sb, xn_tiles.pop(c))

                def emit_pool(c):
                    A_sb, xn_sb = pend.pop(c)
                    last = c == NB - 1
                    for c0, cw in GRP:
                        nc.tensor.matmul(
                            pooled_ps[0:S, c0 // 128:(c0 + cw) // 128, :],
                            A_sb[:, 0:S], xn_sb[:, c0:c0 + cw], start=False,
                            stop=(last and c0 + cw == D),
                            skip_group_check=True)
                    nc.tensor.matmul(
                        pooled_ps[0:S, 10, 0:2], A_sb[:, 0:S],
                        zo_sb[:, 512:514], start=False, stop=last,
                        skip_group_check=True)

                for c in range(NB):
                    if c == 0:
                        # interleave weights + early xt columns so the PE can
                        # start chunk 0 as soon as the first pairs land
                        pre = min(4 * 128, T_pad)
                        for kb in range(KB):
                            nc.sync.dma_start(
                                out=wwt_sb[:, kb, :],
                                in_=t["wwt"][kb * 128:(kb + 1) * 128, :])
                            nc.sync.dma_start(
                                out=xt_sb[:, kb, 0:pre],
                                in_=t["xt"][kb * 128:(kb + 1) * 128, 0:pre])
                        nc.sync.dma_start(out=vrep_sb[:], in_=t["vr"][:])
                        nc.sync.dma_start(
                            out=m1h_sb[:],
                            in_=t["m1h"].rearrange("p (nb s) -> p nb s", s=S))
                        nc.sync.dma_start(out=bwrep_sb[:], in_=t["bwr"][:])
                        nc.sync.dma_start(
                            out=w1t_sb[:],
                            in_=t["w1t"].rearrange("(kb p) m -> p kb m", p=128))
                        nc.sync.dma_start(out=w2b_sb[:], in_=t["w2r"][:])
                        nc.sync.dma_start(out=b1rep_sb[:], in_=t["b1r"][:])
                    # xt columns for chunk c+4 (chunks 0..3 primed at c==0)
                    if c + 4 < NB:
                        x0 = (c + 4) * 128
                        for kb in range(KB):
                            nc.sync.dma_start(
                                out=xt_sb[:, kb, x0:x0 + 128],
                                in_=t["xt"][kb * 128:(kb + 1) * 128,
                                            x0:x0 + 128])
                    # xn prefetch ahead of pooling use
                    for pc in ([0, 1, 2] if c == 0 else
                               ([c + 2] if c + 2 < NB else [])):
                        xn_sb = xnp.tile([128, D], BF16)
                        nc.sync.dma_start(
                            out=xn_sb[:],
                            in_=t["xn"][pc * 128:(pc + 1) * 128, :])
                        xn_tiles[pc] = xn_sb

                    # deferred stages for older chunks go FIRST in each
                    # engine's FIFO so no op ever waits at a queue head on a
                    # fresh cross-engine dependency:
                    #   ACT: exp(c-2) before tanh(c); DVE: A(c-2) before
                    #   mul(c); PE: pooling(c-3) before H(c)
                    if c >= 2:
                        emit_expA(c - 2)
                    if c >= 3:
                        emit_pool(c - 3)

                    # H matmuls: fp8 DoubleRow over kb pairs (stationary =
                    # X.T pair, reused across the 3 column groups)
                    h_ps = [hps2.tile([128, 512], F32, name="hg0"),
                            hps2.tile([128, 512], F32, name="hg1"),
                            hps1.tile([128, 512], F32, name="hg2")]
                    tcol = slice(c * 128, (c + 1) * 128)
                    if USE_FP8:
                        for kp in range(KB // 2):
                            for g, (c0, cw) in enumerate(GRP):
                                nc.tensor.matmul(
                                    h_ps[g][:, 0:cw],
                                    xt_sb[:, 2 * kp:2 * kp + 2, tcol],
                                    wwt_sb[:, 2 * kp:2 * kp + 2, c0:c0 + cw],
                                    start=(kp == 0), stop=(kp == KB // 2 - 1),
                                    perf_mode=mybir.MatmulPerfMode.DoubleRow)
                    else:
                        for kb in range(KB):
                            for g, (c0, cw) in enumerate(GRP):
                                nc.tensor.matmul(
                                    h_ps[g][:, 0:cw],
                                    xt_sb[:, kb, tcol],
                                    wwt_sb[:, kb, c0:c0 + cw],
                                    start=(kb == 0), stop=(kb == KB - 1))

                    if not bw_zero:
                        for g, (c0, cw) in enumerate(GRP):
                            nc.vector.tensor_add(h_ps[g][:, 0:cw],
                                                 h_ps[g][:, 0:cw],
                                                 bwrep_sb[:, c0:c0 + cw])
                    ht_sb = htp.tile([128, D], BF16)
                    for g in (2, 0, 1):
                        c0, cw = GRP[g]
                        nc.scalar.activation(ht_sb[:, c0:c0 + cw],
                                             h_ps[g][:, 0:cw], AF.Tanh,
                                             scale=1.0 / WSCALE)
                    # multiply + feature-axis reduce -> s [128,1]
                    # (tensor_tensor_reduce aborts on HW; use mul + reduce)
                    prod_sb = prodp.tile([128, D], BF16)
                    s_sb = sp_pool.tile([128, 1], F32)
                    nc.vector.tensor_mul(prod_sb[:], ht_sb[:], vrep_sb[:])
                    nc.vector.tensor_reduce(s_sb[:], prod_sb[:],
                                            axis=mybir.AxisListType.X,
                                            op=ALU.add)
                    spend[c] = s_sb

                emit_expA(NB - 2)
                emit_expA(NB - 1)
                emit_pool(NB - 3)
                emit_pool(NB - 2)
                emit_pool(NB - 1)

            # ---- tail / logits head ----
            zc_sb = cp.tile([SP, 1], F32)
            nc.vector.tensor_scalar_max(zc_sb[:], pooled_ps[0:SP, 10, 0:1],
                                        1e-30)
            rz_sb = cp.tile([SP, 1], F32)
            nc.vector.reciprocal(rz_sb[:], zc_sb[:])

            identS = cp.tile([SP, SP], F32)
            make_identity(nc, identS[:])
            pTf_sb = cp.tile([128, KB, SP], F32)
            if TAIL_PE:
                psrc_sb = cp.tile([SP, KB, 128], F32)
                for a0, a1 in ((0, 4), (4, 8), (8, 10)):
                    if b1_zero:
                        nc.scalar.copy(psrc_sb[:, a0:a1, :],
                                       pooled_ps[0:SP, a0:a1, :])
                    else:
                        nc.vector.tensor_scalar_mul(
                            psrc_sb[:, a0:a1, :],
                            pooled_ps[0:SP, a0:a1, :], rz_sb[:])
            elif b1_zero:
                # relu commutes with the positive per-segment 1/z scale when
                # b1 == 0: head runs on UNNORMALIZED pooled, rz applied last.
                # DVE 32x32 stream-transposes, PSUM -> SBUF direct
                # (same-dtype f32; cast to bf16 in one ACT copy after).
                for t_ in range(KB):
                    for a in range(4):
                        nc.vector.transpose(
                            pTf_sb[32 * a:32 * a + 32, t_, :],
                            pooled_ps[0:SP, t_, 32 * a:32 * a + 32])
            else:
                pn_sb = cp.tile([SP, KB, 128], F32)
                nc.vector.tensor_scalar_mul(
                    pn_sb[:, :, :], pooled_ps[0:SP, 0:KB, :], rz_sb[:])
                for t_ in range(KB):
                    for a in range(4):
                        nc.vector.transpose(
                            pTf_sb[32 * a:32 * a + 32, t_, :],
                            pn_sb[0:SP, t_, 32 * a:32 * a + 32])
            pT_sb = cp.tile([128, KB, SP], BF16)
            if not TAIL_PE:
                nc.scalar.copy(pT_sb[:, :, :], pTf_sb[:, :, :])

        if TAIL_PE:
            with tc.tile_pool(name="ptps", bufs=2, space="PSUM") as ptp:
                for db in range(KB):
                    pT_ps = ptp.tile([128, SP], F32)
                    nc.tensor.transpose(pT_ps[:], psrc_sb[:, db, :], identS[:])
                    nc.scalar.copy(pT_sb[:, db, 0:SP], pT_ps[:])
        with tc.tile_pool(name="headps", bufs=1, space="PSUM") as headp:
            hn_ps = headp.tile([SP, DH], F32)
            for db in range(KB):
                nc.tensor.matmul(hn_ps[:], pT_sb[:, db, 0:SP],
                                 w1t_sb[:, db, :],
                                 start=(db == 0), stop=(db == KB - 1))
            if not b1_zero:
                nc.vector.tensor_add(hn_ps[:], hn_ps[:], b1rep_sb[:])
            hn_sb = cp.tile([SP, DH], BF16)
            nc.scalar.activation(hn_sb[:], hn_ps[:], AF.Relu)
            prodh_sb = cp.tile([SP, DH], BF16)
            oraw_sb = cp.tile([SP, 1], F32)
            nc.vector.tensor_mul(prodh_sb[:], hn_sb[:], w2b_sb[:])
            nc.vector.tensor_reduce(oraw_sb[:], prodh_sb[:],
                                    axis=mybir.AxisListType.X, op=ALU.add)
            if b1_zero:
                oval_sb = cp.tile([SP, 1], F32)
                nc.vector.tensor_mul(oval_sb[:], oraw_sb[:], rz_sb[:])
            else:
                oval_sb = oraw_sb
            o_ps = headp.tile([1, SP], F32)
            nc.tensor.transpose(o_ps[:], oval_sb[:], identS[:])
            out_sb = cp.tile([1, SP], F32)
            nc.scalar.copy(out_sb[:], o_ps[:])
            nc.sync.dma_start(out=t["out"][:], in_=out_sb[0:1, 0:S])


def _build_program(T_pad: int, S: int, b1_zero: bool, bw_zero: bool):
    key = (T_pad, S, b1_zero, bw_zero, USE_FP8, TAIL_PE)
    if key in _PROGRAM_CACHE:
        return _PROGRAM_CACHE[key]
    NB = T_pad // 128
    mmdt = FP8 if USE_FP8 else BF16
    nc = bacc.Bacc("TRN2", target_bir_lowering=False, debug=False,
                   num_devices=N_CORES)
    t = {
        "xt": nc.dram_tensor("xt", [D, T_pad], mmdt, kind="ExternalInput").ap(),
        "xn": nc.dram_tensor("xn", [T_pad, D], BF16, kind="ExternalInput").ap(),
        "wwt": nc.dram_tensor("wwt", [D, D], mmdt, kind="ExternalInput").ap(),
        "m1h": nc.dram_tensor("m1h", [128, NB * S], BF16,
                              kind="ExternalInput").ap(),
        "vr": nc.dram_tensor("vr", [128, D], BF16, kind="ExternalInput").ap(),
        "bwr": nc.dram_tensor("bwr", [128, D], F32,
                              kind="ExternalInput").ap(),
        "zo": nc.dram_tensor("zo", [128, 514], BF16, kind="ExternalInput").ap(),
        "w1t": nc.dram_tensor("w1t", [D, DH], BF16, kind="ExternalInput").ap(),
        "b1r": nc.dram_tensor("b1r", [SP, DH], F32, kind="ExternalInput").ap(),
        "w2r": nc.dram_tensor("w2r", [SP, DH], BF16,
                              kind="ExternalInput").ap(),
        "out": nc.dram_tensor("out", [1, S], F32, kind="ExternalOutput").ap(),
    }
    with tile.TileContext(nc) as tc:
        _emit(tc, t, T_pad, S, b1_zero, bw_zero)
    nc.compile()
    _PROGRAM_CACHE[key] = nc
    return nc


def kernel(features, Ww, bw, v, W1, b1, W2, b2, segment_ids):
    global LAST_RESULTS
    features = np.ascontiguousarray(np.asarray(features, dtype=np.float32))
    Ww = np.asarray(Ww, dtype=np.float32)
    bw = np.asarray(bw, dtype=np.float32)
    v = np.asarray(v, dtype=np.float32)
    W1 = np.asarray(W1, dtype=np.float32)
    b1 = np.asarray(b1, dtype=np.float32)
    W2 = np.asarray(W2, dtype=np.float32)
    b2 = np.asarray(b2, dtype=np.float32)
    segment_ids = np.asarray(segment_ids)

    seg64 = segment_ids.astype(np.int64)
    lengths = np.bincount(seg64, minlength=N_SEG)[:N_SEG]
    cuts = _partition_segments(lengths)
    seg_prefix = np.concatenate([[0], np.cumsum(lengths)])
    tok_cuts = [int(seg_prefix[c]) for c in cuts]

    S = max(cuts[c + 1] - cuts[c] for c in range(N_CORES))
    assert S <= SP, f"segments per core {S} exceeds {SP}"
    T_max = max(tok_cuts[c + 1] - tok_cuts[c] for c in range(N_CORES))
    T_pad = max(512, ((T_max + 127) // 128) * 128)
    NB = T_pad // 128

    b1_zero = bool(np.all(b1 == 0))
    bw_zero = bool(np.all(bw == 0))

    mmnp = NPF8 if USE_FP8 else NPBF
    wsc = WSCALE if USE_FP8 else 1.0
    wwt = np.ascontiguousarray((Ww.T * wsc)).astype(mmnp)      # [k, m]
    vr = np.tile(v.reshape(1, D), (128, 1)).astype(NPBF)
    bwr = np.tile((bw.reshape(1, D) * wsc), (128, 1)).astype(np.float32)
    zo = np.zeros((128, 514), dtype=NPBF)
    zo[:, 512:514] = 1.0
    w1t = np.ascontiguousarray(W1.T).astype(NPBF)              # [k, h]
    b1r = np.tile(b1.reshape(1, DH), (SP, 1)).astype(np.float32)
    w2r = np.tile(W2[0:1, :], (SP, 1)).astype(NPBF)

    in_maps = []
    for c in range(N_CORES):
        s0, s1 = cuts[c], cuts[c + 1]
        t0, t1 = tok_cuts[c], tok_cuts[c + 1]
        Tc = t1 - t0
        xn = np.zeros((T_pad, D), dtype=np.float32)
        xn[:Tc] = features[t0:t1]
        xt = np.ascontiguousarray(xn.T).astype(mmnp)
        oh = np.zeros((T_pad, S), dtype=np.float32)
        if Tc > 0:
            loc = seg64[t0:t1] - s0
            ok = (loc >= 0) & (loc < S)
            oh[np.arange(Tc)[ok], loc[ok]] = 1.0
        m1h = np.ascontiguousarray(
            oh.reshape(NB, 128, S).transpose(1, 0, 2).reshape(128, NB * S)
        ).astype(NPBF)
        in_maps.append({
            "xt": xt, "xn": xn.astype(NPBF), "m1h": m1h,
            "wwt": wwt, "vr": vr, "bwr": bwr, "zo": zo,
            "w1t": w1t, "b1r": b1r, "w2r": w2r,
        })

    nc = _build_program(T_pad, S, b1_zero, bw_zero)
    trace = bool(int(os.environ.get("KERNEL_TRACE", "0")))
    res = run_bass_kernel_spmd(nc, in_maps, core_ids=list(range(N_CORES)),
                               trace=trace)
    LAST_RESULTS = res

    out = np.zeros((N_SEG, 1), dtype=np.float32)
    for c in range(N_CORES):
        s0, s1 = cuts[c], cuts[c + 1]
        out[s0:s1, 0] = res.results[c]["out"][0, :s1 - s0]
    if not b1_zero:
        # general path computed h @ W2 unnormalized-free already; rz folded
        # on device only in b1_zero mode. Here z-normalization happened on
        # device (pooled scaled before head), nothing to do.
        pass
    out[:, 0] += b2[0]

    # empty segments: pooled = 0 -> out = relu(b1) @ W2.T + b2 (host patch;
    # device row may be garbage from 0 * (1/0))
    empty = lengths == 0
    if empty.any():
        out[empty, 0] = float(np.maximum(b1, 0.0) @ W2[0] + b2[0])
    return out


# revision 13
# speedup vs baseline: 1.6387x; 1.6387x over previous
"""Trainium2 Bass kernel for nn_ATTEfficient (ragged segment attention pooling).

reference:
    H = tanh(features @ Ww.T + bw)          # [TOTAL, D]
    s = H @ v                                # [TOTAL]
    att = segment_softmax(s, segment_ids)    # [TOTAL]
    pooled = segment_sum(features * att)     # [N_SEG, D]
    h = relu(pooled @ W1.T + b1)             # [N_SEG, D_HEAD]
    out = h @ W2.T + b2                      # [N_SEG, 1]

Sharding: segments (and their contiguous token ranges) split across the 8
cores data-parallel, balanced by token count; weights replicated; all
segment reductions local to one core.

Device pipeline per core, token-partition ("natural H") layout, one
128-token chunk at a time:
    - H chunk [128 tok, 1280] = X.T-pair stationary (fp8 DoubleRow, W
      pre-scaled x64 on host) x Ww.T moving, accumulated in 3 PSUM column
      groups; ACT tanh (scale 1/64) -> ht bf16
    - s row via ONE fused DVE tensor_tensor_reduce (ht * v-broadcast,
      reduce over features) -> per-token s in partition layout, ACT exp
      (no max subtraction: |s| is O(1), exp cannot overflow)
    - A = onehot_mask * e (DVE), pooling matmuls (bf16) accumulate
      pooled[seg,:]+z across all chunks into one 3-bank PSUM group
  tail: z clamp/recip, DVE 32x32 stream-transposes of pooled (PSUM->SBUF
  direct, no PE transposes), 10 bf16 head matmuls, relu, fused DVE dot
  with W2, 1/z scale, out written token-partition [S,1] (host reshapes).

fp8 is e4m3 on X and 64*Ww only (end-to-end rel err ~1.5e-2 measured in
simulation vs the 2e-2 gate); pooling/head stay bf16.

PSUM accumulators are initialized by zero-matmuls (start=True writing the
whole used range) so subsequent accumulating matmuls are order-free.
"""

import os
import numpy as np
import ml_dtypes

import concourse.bass as bass
import concourse.tile as tile
from concourse import bacc, mybir
from concourse.bass_utils import run_bass_kernel_spmd
from concourse.masks import make_identity

F32 = mybir.dt.float32
BF16 = mybir.dt.bfloat16
FP8 = mybir.dt.float8e4
AF = mybir.ActivationFunctionType
ALU = mybir.AluOpType
NPF8 = ml_dtypes.float8_e4m3
NPBF = ml_dtypes.bfloat16

N_CORES = 8
N_SEG = 128
D = 1280
KB = D // 128   # 10 feature blocks
DH = 512
SP = 32         # padded segment-partition count (DVE transpose block)
WSCALE = 64.0   # fp8 weight pre-scale (undone by ACT tanh input scale)

USE_FP8 = bool(int(os.environ.get("KERNEL_FP8", "1")))  # False -> bf16 H
TAIL_PE = bool(int(os.environ.get("KERNEL_TAIL_PE", "1")))  # PE-transpose tail

LAST_RESULTS = None
_PROGRAM_CACHE = {}


def _partition_segments(lengths: np.ndarray) -> list[int]:
    """Split N_SEG contiguous segments into N_CORES contiguous groups
    minimizing the max token count (binary search + greedy packing)."""
    lengths = lengths.astype(np.int64)
    total = int(lengths.sum())

    def cuts_for(cap):
        cuts = [0]
        cur = 0
        for i, L in enumerate(lengths):
            if cur + L > cap and cur > 0:
                cuts.append(i)
                cur = 0
                if len(cuts) > N_CORES:
                    return None
            cur += int(L)
        while len(cuts) < N_CORES:
            cuts.append(N_SEG)
        cuts.append(N_SEG)
        return cuts

    lo, hi = max(int(lengths.max()), (total + N_CORES - 1) // N_CORES), total
    while lo < hi:
        mid = (lo + hi) // 2
        if cuts_for(mid) is not None:
            hi = mid
        else:
            lo = mid + 1
    return cuts_for(lo)


def _emit(tc: tile.TileContext, t: dict, T_pad: int, S: int,
          b1_zero: bool, bw_zero: bool):
    nc = tc.nc
    NB = T_pad // 128
    GRP = ((0, 512), (512, 512), (1024, 256))  # H psum column groups
    mmdt = FP8 if USE_FP8 else BF16

    with tc.tile_pool(name="const", bufs=1) as cp:
        zo_sb = cp.tile([128, 514], BF16)
        nc.sync.dma_start(out=zo_sb[:], in_=t["zo"][:])
        wwt_sb = cp.tile([128, KB, D], mmdt)
        xt_sb = cp.tile([128, KB, T_pad], mmdt)
        vrep_sb = cp.tile([128, D], BF16)
        m1h_sb = cp.tile([128, NB, S], BF16)
        e_sb = cp.tile([128, NB], F32)
        w1t_sb = cp.tile([128, KB, DH], BF16)
        w2b_sb = cp.tile([SP, DH], BF16)
        bwrep_sb = cp.tile([128, D], F32)
        b1rep_sb = cp.tile([SP, DH], F32)

        with tc.tile_pool(name="accps", bufs=1, space="PSUM") as accp:
            # pooled accumulator: blocks 0:10 pooled (d), block 10 cols
            # 0:2 hold z
            pooled_ps = accp.tile([SP, 12, 128], F32)

            # zero-matmuls: order-safe accumulator init + HAM warm while the
            # startup weight/data DMAs stream in
            for _rep in range(6):
                for a0 in (0, 4, 8):
                    nc.tensor.matmul(pooled_ps[:, a0:a0 + 4, :],
                                     zo_sb[:, 0:SP], zo_sb[:, 0:512],
                                     start=True, stop=False,
                                     skip_group_check=True)

            with tc.tile_pool(name="hps2", bufs=2, space="PSUM") as hps2, \
                 tc.tile_pool(name="hps1", bufs=1, space="PSUM") as hps1, \
                 tc.tile_pool(name="xnp", bufs=7) as xnp, \
                 tc.tile_pool(name="htp", bufs=2) as htp, \
                 tc.tile_pool(name="prodp", bufs=2) as prodp, \
                 tc.tile_pool(name="sp", bufs=4) as sp_pool, \
                 tc.tile_pool(name="ap", bufs=3) as ap_pool, \
                 tc.tile_pool(name="xnp2", bufs=1) as _unused:

                xn_tiles = {}
                pend = {}
                spend = {}

                def emit_expA(c):
                    s_sb = spend.pop(c)
                    nc.scalar.activation(e_sb[:, c:c + 1], s_sb[:], AF.Exp)
                    A_sb = ap_pool.tile([128, S], BF16)
                    nc.vector.tensor_scalar_mul(
                        A_sb[:], m1h_sb[:, c, 0:S], e_sb[:, c:c + 1])
                    pend[c] = (A_sb, xn_tiles.pop(c))

                def emit_pool(c):
                    A_sb, xn_sb = pend.pop(c)
                    last = c == NB - 1
                    for c0, cw in GRP:
                        nc.tensor.matmul(
                            pooled_ps[0:S, c0 // 128:(c0 + cw) // 128, :],
                            A_sb[:, 0:S], xn_sb[:, c0:c0 + cw], start=False,
                            stop=(last and c0 + cw == D),
                            skip_group_check=True)
                    nc.tensor.matmul(
                        pooled_ps[0:S, 10, 0:2], A_sb[:, 0:S],
                        zo_sb[:, 512:514], start=False, stop=last,
                        skip_group_check=True)

                for c in range(NB):
                    if c == 0:
                        # interleave weights + early xt columns so the PE can
                        # start chunk 0 as soon as the first pairs land
                        pre = min(4 * 128, T_pad)
                        for kb in range(KB):
                            nc.sync.dma_start(
                                out=wwt_sb[:, kb, :],
                                in_=t["wwt"][kb * 128:(kb + 1) * 128, :])
                            nc.sync.dma_start(
                                out=xt_sb[:, kb, 0:pre],
                                in_=t["xt"][kb * 128:(kb + 1) * 128, 0:pre])
                        nc.sync.dma_start(out=vrep_sb[:], in_=t["vr"][:])
                        nc.sync.dma_start(
                            out=m1h_sb[:],
                            in_=t["m1h"].rearrange("p (nb s) -> p nb s", s=S))
                        nc.sync.dma_start(out=bwrep_sb[:], in_=t["bwr"][:])
                        nc.sync.dma_start(
                            out=w1t_sb[:],
                            in_=t["w1t"].rearrange("(kb p) m -> p kb m", p=128))
                        nc.sync.dma_start(out=w2b_sb[:], in_=t["w2r"][:])
                        nc.sync.dma_start(out=b1rep_sb[:], in_=t["b1r"][:])
                    # xt columns prefetch: one grouped DMA per 4 chunks
                    # (chunks 0..3 primed at c==0), ~4-chunk lead
                    if c % 4 == 0 and c + 4 < NB:
                        x0 = (c + 4) * 128
                        x1 = min(x0 + 512, T_pad)
                        nc.sync.dma_start(
                            out=xt_sb[:, :, x0:x1],
                            in_=t["xt"][:, x0:x1]
                                .rearrange("(kb p) n -> p kb n", p=128))
                    # xn prefetch ahead of pooling use
                    for pc in ([0, 1, 2] if c == 0 else
                               ([c + 2] if c + 2 < NB else [])):
                        xn_sb = xnp.tile([128, D], BF16)
                        nc.sync.dma_start(
                            out=xn_sb[:],
                            in_=t["xn"][pc * 128:(pc + 1) * 128, :])
                        xn_tiles[pc] = xn_sb

                    # deferred stages for older chunks go FIRST in each
                    # engine's FIFO so no op ever waits at a queue head on a
                    # fresh cross-engine dependency:
                    #   ACT: exp(c-2) before tanh(c); DVE: A(c-2) before
                    #   mul(c); PE: pooling(c-3) before H(c)
                    if c >= 2:
                        emit_expA(c - 2)
                    if c >= 3:
                        emit_pool(c - 3)

                    # H matmuls: fp8 DoubleRow over kb pairs (stationary =
                    # X.T pair, reused across the 3 column groups)
                    h_ps = [hps2.tile([128, 512], F32, name="hg0"),
                            hps2.tile([128, 512], F32, name="hg1"),
                            hps1.tile([128, 512], F32, name="hg2")]
                    tcol = slice(c * 128, (c + 1) * 128)
                    if USE_FP8:
                        for kp in range(KB // 2):
                            for g, (c0, cw) in enumerate(GRP):
                                nc.tensor.matmul(
                                    h_ps[g][:, 0:cw],
                                    xt_sb[:, 2 * kp:2 * kp + 2, tcol],
                                    wwt_sb[:, 2 * kp:2 * kp + 2, c0:c0 + cw],
                                    start=(kp == 0), stop=(kp == KB // 2 - 1),
                                    perf_mode=mybir.MatmulPerfMode.DoubleRow)
                    else:
                        for kb in range(KB):
                            for g, (c0, cw) in enumerate(GRP):
                                nc.tensor.matmul(
                                    h_ps[g][:, 0:cw],
                                    xt_sb[:, kb, tcol],
                                    wwt_sb[:, kb, c0:c0 + cw],
                                    start=(kb == 0), stop=(kb == KB - 1))

                    if not bw_zero:
                        for g, (c0, cw) in enumerate(GRP):
                            nc.vector.tensor_add(h_ps[g][:, 0:cw],
                                                 h_ps[g][:, 0:cw],
                                                 bwrep_sb[:, c0:c0 + cw])
                    ht_sb = htp.tile([128, D], BF16)
                    for g in (2, 0, 1):
                        c0, cw = GRP[g]
                        nc.scalar.activation(ht_sb[:, c0:c0 + cw],
                                             h_ps[g][:, 0:cw], AF.Tanh,
                                             scale=1.0 / WSCALE)
                    # multiply + feature-axis reduce -> s [128,1]
                    # (tensor_tensor_reduce aborts on HW; use mul + reduce)
                    prod_sb = prodp.tile([128, D], BF16)
                    s_sb = sp_pool.tile([128, 1], F32)
                    nc.vector.tensor_mul(prod_sb[:], ht_sb[:], vrep_sb[:])
                    nc.vector.tensor_reduce(s_sb[:], prod_sb[:],
                                            axis=mybir.AxisListType.X,
                                            op=ALU.add)
                    spend[c] = s_sb

                emit_expA(NB - 2)
                emit_expA(NB - 1)
                emit_pool(NB - 3)
                emit_pool(NB - 2)
                emit_pool(NB - 1)

            # ---- tail / logits head ----
            zc_sb = cp.tile([SP, 1], F32)
            nc.vector.tensor_scalar_max(zc_sb[:], pooled_ps[0:SP, 10, 0:1],
                                        1e-30)
            rz_sb = cp.tile([SP, 1], F32)
            nc.vector.reciprocal(rz_sb[:], zc_sb[:])

            identS = cp.tile([SP, SP], F32)
            make_identity(nc, identS[:])
            pTf_sb = cp.tile([128, KB, SP], F32)
            if TAIL_PE:
                psrc_sb = cp.tile([SP, KB, 128], F32)
                for a0, a1 in ((0, 4), (4, 8), (8, 10)):
                    if b1_zero:
                        nc.scalar.copy(psrc_sb[:, a0:a1, :],
                                       pooled_ps[0:SP, a0:a1, :])
                    else:
                        nc.vector.tensor_scalar_mul(
                            psrc_sb[:, a0:a1, :],
                            pooled_ps[0:SP, a0:a1, :], rz_sb[:])
            elif b1_zero:
                # relu commutes with the positive per-segment 1/z scale when
                # b1 == 0: head runs on UNNORMALIZED pooled, rz applied last.
                # DVE 32x32 stream-transposes, PSUM -> SBUF direct
                # (same-dtype f32; cast to bf16 in one ACT copy after).
                for t_ in range(KB):
                    for a in range(4):
                        nc.vector.transpose(
                            pTf_sb[32 * a:32 * a + 32, t_, :],
                            pooled_ps[0:SP, t_, 32 * a:32 * a + 32])
            else:
                pn_sb = cp.tile([SP, KB, 128], F32)
                nc.vector.tensor_scalar_mul(
                    pn_sb[:, :, :], pooled_ps[0:SP, 0:KB, :], rz_sb[:])
                for t_ in range(KB):
                    for a in range(4):
                        nc.vector.transpose(
                            pTf_sb[32 * a:32 * a + 32, t_, :],
                            pn_sb[0:SP, t_, 32 * a:32 * a + 32])
            pT_sb = cp.tile([128, KB, SP], BF16)
            if not TAIL_PE:
                nc.scalar.copy(pT_sb[:, :, :], pTf_sb[:, :, :])

        if TAIL_PE:
            with tc.tile_pool(name="ptps", bufs=2, space="PSUM") as ptp:
                for db in range(KB):
                    pT_ps = ptp.tile([128, SP], F32)
                    nc.tensor.transpose(pT_ps[:], psrc_sb[:, db, :], identS[:])
                    nc.scalar.copy(pT_sb[:, db, 0:SP], pT_ps[:])
        with tc.tile_pool(name="headps", bufs=1, space="PSUM") as headp:
            hn_ps = headp.tile([SP, DH], F32)
            for db in range(KB):
                nc.tensor.matmul(hn_ps[:], pT_sb[:, db, 0:SP],
                                 w1t_sb[:, db, :],
                                 start=(db == 0), stop=(db == KB - 1))
            if not b1_zero:
                nc.vector.tensor_add(hn_ps[:], hn_ps[:], b1rep_sb[:])
            hn_sb = cp.tile([SP, DH], BF16)
            nc.scalar.activation(hn_sb[:], hn_ps[:], AF.Relu)
            prodh_sb = cp.tile([SP, DH], BF16)
            oraw_sb = cp.tile([SP, 1], F32)
            nc.vector.tensor_mul(prodh_sb[:], hn_sb[:], w2b_sb[:])
            nc.vector.tensor_reduce(oraw_sb[:], prodh_sb[:],
                                    axis=mybir.AxisListType.X, op=ALU.add)
            if b1_zero:
                oval_sb = cp.tile([SP, 1], F32)
                nc.vector.tensor_mul(oval_sb[:], oraw_sb[:], rz_sb[:])
            else:
                oval_sb = oraw_sb
            o_ps = headp.tile([1, SP], F32)
            nc.tensor.transpose(o_ps[:], oval_sb[:], identS[:])
            out_sb = cp.tile([1, SP], F32)
            nc.scalar.copy(out_sb[:], o_ps[:])
            nc.sync.dma_start(out=t["out"][:], in_=out_sb[0:1, 0:S])


def _build_program(T_pad: int, S: int, b1_zero: bool, bw_zero: bool):
    key = (T_pad, S, b1_zero, bw_zero, USE_FP8, TAIL_PE)
    if key in _PROGRAM_CACHE:
        return _PROGRAM_CACHE[key]
    NB = T_pad // 128
    mmdt = FP8 if USE_FP8 else BF16
    nc = bacc.Bacc("TRN2", target_bir_lowering=False, debug=False,
                   num_devices=N_CORES)
    t = {
        "xt": nc.dram_tensor("xt", [D, T_pad], mmdt, kind="ExternalInput").ap(),
        "xn": nc.dram_tensor("xn", [T_pad, D], BF16, kind="ExternalInput").ap(),
        "wwt": nc.dram_tensor("wwt", [D, D], mmdt, kind="ExternalInput").ap(),
        "m1h": nc.dram_tensor("m1h", [128, NB * S], BF16,
                              kind="ExternalInput").ap(),
        "vr": nc.dram_tensor("vr", [128, D], BF16, kind="ExternalInput").ap(),
        "bwr": nc.dram_tensor("bwr", [128, D], F32,
                              kind="ExternalInput").ap(),
        "zo": nc.dram_tensor("zo", [128, 514], BF16, kind="ExternalInput").ap(),
        "w1t": nc.dram_tensor("w1t", [D, DH], BF16, kind="ExternalInput").ap(),
        "b1r": nc.dram_tensor("b1r", [SP, DH], F32, kind="ExternalInput").ap(),
        "w2r": nc.dram_tensor("w2r", [SP, DH], BF16,
                              kind="ExternalInput").ap(),
        "out": nc.dram_tensor("out", [1, S], F32, kind="ExternalOutput").ap(),
    }
    with tile.TileContext(nc) as tc:
        _emit(tc, t, T_pad, S, b1_zero, bw_zero)
    nc.compile()
    _PROGRAM_CACHE[key] = nc
    return nc


def kernel(features, Ww, bw, v, W1, b1, W2, b2, segment_ids):
    global LAST_RESULTS
    features = np.ascontiguousarray(np.asarray(features, dtype=np.float32))
    Ww = np.asarray(Ww, dtype=np.float32)
    bw = np.asarray(bw, dtype=np.float32)
    v = np.asarray(v, dtype=np.float32)
    W1 = np.asarray(W1, dtype=np.float32)
    b1 = np.asarray(b1, dtype=np.float32)
    W2 = np.asarray(W2, dtype=np.float32)
    b2 = np.asarray(b2, dtype=np.float32)
    segment_ids = np.asarray(segment_ids)

    seg64 = segment_ids.astype(np.int64)
    lengths = np.bincount(seg64, minlength=N_SEG)[:N_SEG]
    cuts = _partition_segments(lengths)
    seg_prefix = np.concatenate([[0], np.cumsum(lengths)])
    tok_cuts = [int(seg_prefix[c]) for c in cuts]

    S = max(cuts[c + 1] - cuts[c] for c in range(N_CORES))
    assert S <= SP, f"segments per core {S} exceeds {SP}"
    T_max = max(tok_cuts[c + 1] - tok_cuts[c] for c in range(N_CORES))
    T_pad = max(512, ((T_max + 127) // 128) * 128)
    NB = T_pad // 128

    b1_zero = bool(np.all(b1 == 0))
    bw_zero = bool(np.all(bw == 0))

    mmnp = NPF8 if USE_FP8 else NPBF
    wsc = WSCALE if USE_FP8 else 1.0
    wwt = np.ascontiguousarray((Ww.T * wsc)).astype(mmnp)      # [k, m]
    vr = np.tile(v.reshape(1, D), (128, 1)).astype(NPBF)
    bwr = np.tile((bw.reshape(1, D) * wsc), (128, 1)).astype(np.float32)
    zo = np.zeros((128, 514), dtype=NPBF)
    zo[:, 512:514] = 1.0
    w1t = np.ascontiguousarray(W1.T).astype(NPBF)              # [k, h]
    b1r = np.tile(b1.reshape(1, DH), (SP, 1)).astype(np.float32)
    w2r = np.tile(W2[0:1, :], (SP, 1)).astype(NPBF)

    in_maps = []
    for c in range(N_CORES):
        s0, s1 = cuts[c], cuts[c + 1]
        t0, t1 = tok_cuts[c], tok_cuts[c + 1]
        Tc = t1 - t0
        xn = np.zeros((T_pad, D), dtype=np.float32)
        xn[:Tc] = features[t0:t1]
        xt = np.ascontiguousarray(xn.T).astype(mmnp)
        oh = np.zeros((T_pad, S), dtype=np.float32)
        if Tc > 0:
            loc = seg64[t0:t1] - s0
            ok = (loc >= 0) & (loc < S)
            oh[np.arange(Tc)[ok], loc[ok]] = 1.0
        m1h = np.ascontiguousarray(
            oh.reshape(NB, 128, S).transpose(1, 0, 2).reshape(128, NB * S)
        ).astype(NPBF)
        in_maps.append({
            "xt": xt, "xn": xn.astype(NPBF), "m1h": m1h,
            "wwt": wwt, "vr": vr, "bwr": bwr, "zo": zo,
            "w1t": w1t, "b1r": b1r, "w2r": w2r,
        })

    nc = _build_program(T_pad, S, b1_zero, bw_zero)
    trace = bool(int(os.environ.get("KERNEL_TRACE", "0")))
    res = run_bass_kernel_spmd(nc, in_maps, core_ids=list(range(N_CORES)),
                               trace=trace)
    LAST_RESULTS = res

    out = np.zeros((N_SEG, 1), dtype=np.float32)
    for c in range(N_CORES):
        s0, s1 = cuts[c], cuts[c + 1]
        out[s0:s1, 0] = res.results[c]["out"][0, :s1 - s0]
    if not b1_zero:
        # general path computed h @ W2 unnormalized-free already; rz folded
        # on device only in b1_zero mode. Here z-normalization happened on
        # device (pooled scaled before head), nothing to do.
        pass
    out[:, 0] += b2[0]

    # empty segments: pooled = 0 -> out = relu(b1) @ W2.T + b2 (host patch;
    # device row may be garbage from 0 * (1/0))
    empty = lengths == 0
    if empty.any():
        out[empty, 0] = float(np.maximum(b1, 0.0) @ W2[0] + b2[0])
    return out
